# revision 1
# baseline (speedup 1.0000x reference)
"""Trainium2 Bass kernel for per-token multi-head self-attention.

Computation (per token t):
  q,k,v = x @ W{q,k,v}.T ; scores = (q_t k_t^T)/sqrt(128) over heads [16x16]
  out_t = softmax(scores) @ v_t ; y = out @ Wo.T

Sharding: data-parallel over the 16384 tokens -> 8 cores x 2048 tokens.
All activations flow on-chip in transposed ([feature, token]) layout; the
host pre-transposes x shards and weights so every matmul operand loads
naturally with the contraction dim on partitions (no on-chip transposes for
the 4 big matmuls). fp32r (full-rate tf32-like) for the big matmuls.

Middle stage per 4-token group: per-token 16x16 score matmuls -> exp (ACT)
into a block-diagonal [128,64] attn matrix -> one AV matmul against the
PE-transposed [4tok x 32, d] V block (with a ones column producing the
softmax normalizer Z) -> per-partition 1/Z scale -> PE-transpose back.
"""
import math
from contextlib import ExitStack

import numpy as np

NCORES = 8
E = 2048          # hidden
NH = 16           # heads
HD = 128          # head dim
TPC = 2048        # tokens per core
TC = 512          # token chunk in pass B
P = 128

_cached = {}


def _build_program():
    import concourse.bass as bass
    import concourse.tile as tile
    from concourse import bacc, mybir
    from concourse.masks import make_identity

    f32 = mybir.dt.float32
    f32r = mybir.dt.float32r

    nc = bacc.Bacc("TRN2", target_bir_lowering=False, debug=False)

    xT_d = nc.dram_tensor("xT", [E, TPC], f32r, kind="ExternalInput").ap()
    WqT_d = nc.dram_tensor("WqT", [E, E], f32r, kind="ExternalInput").ap()
    WkT_d = nc.dram_tensor("WkT", [E, E], f32r, kind="ExternalInput").ap()
    WvT_d = nc.dram_tensor("WvT", [E, E], f32r, kind="ExternalInput").ap()
    WoT_d = nc.dram_tensor("WoT", [E, E], f32r, kind="ExternalInput").ap()
    yT_d = nc.dram_tensor("yT", [E, TPC], f32, kind="ExternalOutput").ap()

    qT_d = nc.dram_tensor("qT_scr", [E, TPC], f32).ap()
    kT_d = nc.dram_tensor("kT_scr", [E, TPC], f32).ap()
    vT_d = nc.dram_tensor("vT_scr", [E, TPC], f32).ap()

    NE = E // P   # 16 k-tiles
    NO = E // P   # 16 o-tiles
    SC = 1.0 / math.sqrt(HD)

    with tile.TileContext(nc) as tc, ExitStack() as ctx:
        glob = ctx.enter_context(tc.tile_pool(name="glob", bufs=1))
        ident = glob.tile([P, P], f32)
        make_identity(nc, ident)

        # ============ PASS A: qT/kT/vT = (W @ x.T) -> DRAM ============
        with nc.named_scope("passA"), \
             tc.tile_pool(name="xsb", bufs=1) as xpool, \
             tc.tile_pool(name="wA", bufs=6) as wpool, \
             tc.tile_pool(name="psA", bufs=8, space="PSUM") as pspool, \
             tc.tile_pool(name="stA", bufs=4) as stpool:
            xsb = xpool.tile([P, NE, TPC], f32r)
            for e in range(NE):
                nc.sync.dma_start(out=xsb[:, e, :], in_=xT_d[e * P:(e + 1) * P, :])

            wmats = [WqT_d, WkT_d, WvT_d]
            outs = [qT_d, kT_d, vT_d]
            for oi in range(NO):
                wg = []
                for m in range(3):
                    wt = wpool.tile([P, NE, P], f32r, tag="wA")
                    for e in range(NE):
                        nc.sync.dma_start(
                            out=wt[:, e, :],
                            in_=wmats[m][e * P:(e + 1) * P,
                                         oi * P:(oi + 1) * P])
                    wg.append(wt)
                for tcix in range(TPC // TC):
                    for m in range(3):
                        acc = pspool.tile([P, TC], f32, tag="accA")
                        for e in range(NE):
                            nc.tensor.matmul(
                                acc,
                                wg[m][:, e, :],
                                xsb[:, e, tcix * TC:(tcix + 1) * TC],
                                start=(e == 0), stop=(e == NE - 1))
                        st = stpool.tile([P, TC], f32, tag="stA")
                        nc.vector.tensor_copy(st, acc)
                        nc.sync.dma_start(
                            out=outs[m][oi * P:(oi + 1) * P,
                                        tcix * TC:(tcix + 1) * TC],
                            in_=st)

        import os as _os
        if _os.environ.get("KERNEL_PASS_A_ONLY"):
            # debug: skip pass B entirely (output stays unwritten)
            _skip_b = True
        else:
            _skip_b = False
        # ============ PASS B: attention + Wo ============
        NG = TC // 4           # 4-token groups per chunk
        SUB = 64               # tokens per v2 relayout block
        if _skip_b:
            qkvp = None
        if not _skip_b:
         with nc.named_scope("passB"), \
             tc.tile_pool(name="qkv", bufs=1) as qkvp, \
             tc.tile_pool(name="v2p", bufs=1) as v2p, \
             tc.tile_pool(name="bdp", bufs=1) as bdp, \
             tc.tile_pool(name="vgp", bufs=1) as vgp, \
             tc.tile_pool(name="mid", bufs=4) as mid, \
             tc.tile_pool(name="aop", bufs=2) as aop, \
             tc.tile_pool(name="woP", bufs=2) as woP, \
             tc.tile_pool(name="yst", bufs=3) as yst, \
             tc.tile_pool(name="psS", bufs=2, space="PSUM") as psS, \
             tc.tile_pool(name="psM", bufs=4, space="PSUM") as psM, \
             tc.tile_pool(name="psY", bufs=2, space="PSUM") as psY:

            # persistent manually-rotated slots (stable zero padding)
            NBD = 8
            bd_slots = []
            for i in range(NBD):
                t = bdp.tile([P, 64], f32, tag=f"bd{i}")
                nc.vector.memset(t, 0.0)
                bd_slots.append(t)
            NV2 = 2
            v2_slots = []
            for i in range(NV2):
                t = v2p.tile([P, SUB, 32], f32, tag=f"v2_{i}")
                nc.vector.memset(t, 0.0)
                v2_slots.append(t)
            NVG = 8
            vg_slots = []
            for i in range(NVG):
                t = vgp.tile([P, HD + 1], f32, tag=f"vg{i}")
                nc.vector.memset(t[:, HD:HD + 1], 1.0)
                vg_slots.append(t)

            # Wo matmul stream for chunk c-1, interleaved 2 MMs per middle
            # group of chunk c so the PE never idles long enough to cool.
            wo_seq = [(oi, h) for oi in range(NO) for h in range(NH)]

            def wo_step(state, nsteps):
                for _ in range(nsteps):
                    if state is None or state["pos"] >= len(wo_seq):
                        return
                    oi, h = wo_seq[state["pos"]]
                    state["pos"] += 1
                    if h == 0:
                        wo = woP.tile([P, NH, P], f32r, tag="wo", name="wo")
                        nc.sync.dma_start(
                            out=wo,
                            in_=WoT_d[:, oi * P:(oi + 1) * P]
                            .rearrange("(hh p) o -> p hh o", p=P))
                        state["wo"] = wo
                        state["yp"] = psY.tile([P, TC], f32, tag="yps", name="yps")
                    nc.tensor.matmul(
                        state["yp"], state["wo"][:, h, :],
                        state["aoT"][:, h, :],
                        start=(h == 0), stop=(h == NH - 1))
                    if h == NH - 1:
                        ys = yst.tile([P, TC], f32, tag="ys")
                        nc.vector.tensor_copy(ys, state["yp"])
                        nc.sync.dma_start(
                            out=yT_d[oi * P:(oi + 1) * P,
                                     state["t0"]:state["t0"] + TC],
                            in_=ys)

            gi_all = 0
            v2i = 0
            prev = None
            for tcix in range(TPC // TC):
                t0 = tcix * TC
                q_sb = qkvp.tile([P, NH, TC], f32, tag="q")
                k_sb = qkvp.tile([P, NH, TC], f32, tag="k")
                v_sb = qkvp.tile([P, NH, TC], f32, tag="v")
                for g in range(NH):
                    nc.sync.dma_start(out=q_sb[:, g, :],
                                      in_=qT_d[g * P:(g + 1) * P, t0:t0 + TC])
                    nc.sync.dma_start(out=k_sb[:, g, :],
                                      in_=kT_d[g * P:(g + 1) * P, t0:t0 + TC])
                    nc.sync.dma_start(out=v_sb[:, g, :],
                                      in_=vT_d[g * P:(g + 1) * P, t0:t0 + TC])

                aoT = aop.tile([P, NH, TC], f32r, tag="aoT")

                for sub in range(TC // SUB):
                    # relayout v to token-major with padded 32-col slots
                    v2 = v2_slots[v2i % NV2]
                    v2i += 1
                    nc.gpsimd.tensor_copy(
                        v2[:, :, 0:NH],
                        v_sb[:, :, sub * SUB:(sub + 1) * SUB]
                        .rearrange("p g t -> p t g"))

                    for gi4 in range(SUB // 4):
                        tt = sub * SUB + gi4 * 4   # first token in group
                        bd = bd_slots[gi_all % NBD]
                        vg = vg_slots[gi_all % NVG]
                        gi_all += 1

                        # V block transpose: [128, 4*32] -> [4*32, 128]
                        vg_ps = psM.tile([P, P], f32, tag="mps")
                        nc.tensor.transpose(
                            vg_ps,
                            v2[:, gi4 * 4:(gi4 + 1) * 4, :]
                            .rearrange("p t g -> p (t g)"),
                            ident)
                        nc.vector.tensor_copy(vg[:, 0:HD], vg_ps)

                        # scores for 4 tokens -> one psum tile at 32-strips
                        sc_ps = psS.tile([P, NH], f32, tag="scps")
                        for j in range(4):
                            t = tt + j
                            nc.tensor.matmul(
                                sc_ps[32 * j:32 * j + NH, :],
                                k_sb[:, :, t], q_sb[:, :, t],
                                start=True, stop=True,
                                tile_position=(0, 32 * j))
                        # exp for all 4 tokens in one ACT op, then build the
                        # block-diagonal with gpsimd (idle engine) copies
                        es = mid.tile([P, NH], f32, tag="es")
                        nc.scalar.activation(
                            out=es, in_=sc_ps,
                            func=mybir.ActivationFunctionType.Exp,
                            scale=SC)
                        for j in range(4):
                            nc.gpsimd.tensor_copy(
                                bd[32 * j:32 * j + NH, NH * j:NH * (j + 1)],
                                es[32 * j:32 * j + NH, :])

                        # AV: [64,(t,h)] x [128, d+1]
                        av_ps = psM.tile([P, HD + 1], f32, tag="mps")
                        nc.tensor.matmul(av_ps[0:64, :], bd, vg, start=True, stop=True)

                        invz = mid.tile([64, 1], f32, tag="invz")
                        nc.vector.reciprocal(invz, av_ps[0:64, HD:HD + 1])
                        ao = mid.tile([64, HD], f32, tag="ao")
                        nc.vector.tensor_scalar_mul(ao, av_ps[0:64, 0:HD], invz)

                        # transpose back: [64,(t,h) x 128 d] -> [128 d, 64]
                        aoT_ps = psM.tile([P, 64], f32, tag="mps")
                        nc.tensor.transpose(aoT_ps, ao, ident[0:64, 0:64])
                        nc.vector.tensor_copy(
                            aoT[:, :, tt:tt + 4].rearrange("p h t -> p h t"),
                            aoT_ps.rearrange("p (t h) -> p h t", t=4))
                        wo_step(prev, 2)

                # drain any remainder of the previous chunk's Wo stream
                wo_step(prev, len(wo_seq))
                prev = {"pos": 0, "aoT": aoT, "t0": t0, "wo": None, "yp": None}
            wo_step(prev, len(wo_seq))

    nc.compile()
    return nc


def _get_program():
    if "nc" not in _cached:
        _cached["nc"] = _build_program()
    return _cached["nc"]


def kernel(x, Wq, Wk, Wv, Wo):
    from concourse.bass_utils import run_bass_kernel_spmd

    B, S, H = x.shape
    assert (B * S, H) == (NCORES * TPC, E)
    nc = _get_program()

    xf = np.ascontiguousarray(x.reshape(B * S, H))
    WqT = np.ascontiguousarray(Wq.T)
    WkT = np.ascontiguousarray(Wk.T)
    WvT = np.ascontiguousarray(Wv.T)
    WoT = np.ascontiguousarray(Wo.T)

    in_maps = []
    for i in range(NCORES):
        xT = np.ascontiguousarray(xf[i * TPC:(i + 1) * TPC, :].T)
        in_maps.append({"xT": xT, "WqT": WqT, "WkT": WkT,
                        "WvT": WvT, "WoT": WoT})

    import os
    trace = bool(int(os.environ.get("BASS_KERNEL_TRACE", "0")))
    res = run_bass_kernel_spmd(nc, in_maps, core_ids=list(range(NCORES)),
                               trace=trace)
    if trace:
        _cached["last_results"] = res
    parts = [res.results[i]["yT"].T for i in range(NCORES)]
    y = np.concatenate(parts, axis=0).reshape(B, S, H)
    return np.ascontiguousarray(y.astype(np.float32))



# revision 10
# speedup vs baseline: 2.2744x; 2.2744x over previous
"""Trainium2 Bass kernel for per-token multi-head self-attention (v3).

Computation (per token t):
  q,k,v = x @ W{q,k,v}.T ; scores = (q_t k_t^T)/sqrt(128) over heads [16x16]
  out_t = softmax(scores) @ v_t ; y = out @ Wo.T

Sharding: data-parallel over the 16384 tokens -> 8 cores x 2048 tokens.

v3 design (vs v2 baseline):
  * bf16 everywhere on-chip (fp32 tiny matmuls were 4 cyc/row + double
    instruction count; bf16 is 1 cyc/row).
  * 8-token score groups: ONE [128,128] matmul computes all 64 16x16
    token-score blocks' worth for 8 tokens (cross-token products land
    off-block-diagonal and are zeroed by a precomputed mask on gpsimd),
    so the exp'd+masked tile IS the block-diagonal AV stationary with no
    per-token gpsimd copies.
  * V is produced token-major (v_nat[T,E]) in pass A via PE transposes,
    so pass B loads the AV moving operand [(t,g), d] directly by DMA.
  * Wo matmuls for chunk c-1 are interleaved into chunk c's group loop
    to keep the PE continuously busy (pstate ramp: 2.4 GHz needs ~3us of
    uninterrupted PE activity).
"""
import math
from contextlib import ExitStack

import numpy as np

NCORES = 8
E = 2048          # hidden
NH = 16           # heads
HD = 128          # head dim
TPC = 2048        # tokens per core
TC = 512          # token chunk in pass B
P = 128
GS = 8            # tokens per score group
NG = TC // GS     # groups per chunk (64)

_cached = {}


def _build_program():
    import concourse.bass as bass
    import concourse.tile as tile
    from concourse import bacc, mybir

    f32 = mybir.dt.float32
    bf16 = mybir.dt.bfloat16
    AOP = mybir.AluOpType

    nc = bacc.Bacc("TRN2", target_bir_lowering=False, debug=False)

    xT_d = nc.dram_tensor("xT", [E, TPC], bf16, kind="ExternalInput").ap()
    WqT_d = nc.dram_tensor("WqT", [E, E], bf16, kind="ExternalInput").ap()
    WkT_d = nc.dram_tensor("WkT", [E, E], bf16, kind="ExternalInput").ap()
    WvT_d = nc.dram_tensor("WvT", [E, E], bf16, kind="ExternalInput").ap()
    WoT_d = nc.dram_tensor("WoT", [E, E], bf16, kind="ExternalInput").ap()
    ident_d = nc.dram_tensor("ident", [P, P], bf16, kind="ExternalInput").ap()
    mask_d = nc.dram_tensor("mask", [P, P], bf16, kind="ExternalInput").ap()
    yT_d = nc.dram_tensor("yT", [E, TPC], f32, kind="ExternalOutput").ap()

    qT_d = nc.dram_tensor("qT_scr", [E, TPC], bf16).ap()
    kT_d = nc.dram_tensor("kT_scr", [E, TPC], bf16).ap()
    vnat_d = nc.dram_tensor("vnat_scr", [TPC, E], bf16).ap()

    NE = E // P   # 16 k-tiles
    NO = E // P   # 16 o-tiles
    SC = 1.0 / math.sqrt(HD)

    with tile.TileContext(nc) as tc, ExitStack() as ctx:
        glob = ctx.enter_context(tc.tile_pool(name="glob", bufs=1))
        ident = glob.tile([P, P], bf16)
        nc.sync.dma_start(out=ident, in_=ident_d)
        maskt = glob.tile([P, P], bf16)
        nc.sync.dma_start(out=maskt, in_=mask_d)

        # ============ PASS A: qT/kT (feature-major) + v_nat (token-major) ====
        with nc.named_scope("passA"), \
             tc.tile_pool(name="xsb", bufs=1) as xpool, \
             tc.tile_pool(name="wA", bufs=2) as wpool, \
             tc.tile_pool(name="psA", bufs=2, space="PSUM") as pspool, \
             tc.tile_pool(name="vtps", bufs=2, space="PSUM") as vtpool, \
             tc.tile_pool(name="stA", bufs=3) as stpool, \
             tc.tile_pool(name="vstA", bufs=2) as vstpool:
            xsb = xpool.tile([P, NE, TPC], bf16)
            for e in range(NE):
                nc.sync.dma_start(out=xsb[:, e, :], in_=xT_d[e * P:(e + 1) * P, :])

            wmats = [WqT_d, WkT_d, WvT_d]
            outs = [qT_d, kT_d, None]
            pend_vst = None  # (vst tile, tc index) awaiting transpose
            for oi in range(NO):
                wg = []
                for m in range(3):
                    wt = wpool.tile([P, NE, P], bf16, tag=f"w{m}", name="wt")
                    nc.sync.dma_start(
                        out=wt,
                        in_=wmats[m][:, oi * P:(oi + 1) * P]
                        .rearrange("(e p) o -> p e o", p=P))
                    wg.append(wt)

                def do_transpose(pend):
                    vst, ptc, poi = pend
                    vt = vtpool.tile([P, 4, P], bf16, tag="vt", name="vt")
                    for j in range(4):
                        nc.tensor.transpose(vt[:, j, :],
                                            vst[:, j * P:(j + 1) * P], ident)
                    vst2 = vstpool.tile([P, 4, P], bf16, tag="vst2",
                                        name="vst2")
                    nc.vector.tensor_copy(vst2, vt)
                    # rows are tokens ptc*TC + j*P + p ; cols poi*P..+P
                    nc.sync.dma_start(
                        out=vnat_d[ptc * TC:(ptc + 1) * TC,
                                   poi * P:(poi + 1) * P]
                        .rearrange("(j p) d -> p j d", j=4),
                        in_=vst2)

                for tcix in range(TPC // TC):
                    for m in range(3):
                        acc = pspool.tile([P, TC], f32, tag="accA", name="acc")
                        for e in range(NE):
                            nc.tensor.matmul(
                                acc,
                                wg[m][:, e, :],
                                xsb[:, e, tcix * TC:(tcix + 1) * TC],
                                start=(e == 0), stop=(e == NE - 1))
                        if m < 2:
                            st = stpool.tile([P, TC], bf16, tag="stA",
                                             name="st")
                            nc.vector.tensor_copy(st, acc)
                            nc.sync.dma_start(
                                out=outs[m][oi * P:(oi + 1) * P,
                                            tcix * TC:(tcix + 1) * TC],
                                in_=st)
                        else:
                            # transposes of the PREVIOUS vst go first: they
                            # must precede the new vst's buffer-slot reuse in
                            # program order, and their input has long been
                            # ready so the PE doesn't stall.
                            if pend_vst is not None:
                                do_transpose(pend_vst)
                            vst = stpool.tile([P, TC], bf16, tag="stA",
                                              name="vst")
                            nc.vector.tensor_copy(vst, acc)
                            pend_vst = (vst, tcix, oi)
            if pend_vst is not None:
                do_transpose(pend_vst)
                pend_vst = None

        # ============ PASS B: attention + Wo ============
        with nc.named_scope("passB"), \
             tc.tile_pool(name="qk", bufs=2) as qkp, \
             tc.tile_pool(name="vgp", bufs=2) as vgp, \
             tc.tile_pool(name="aop", bufs=2) as aop, \
             tc.tile_pool(name="mid", bufs=4) as mid, \
             tc.tile_pool(name="woP", bufs=2) as woP, \
             tc.tile_pool(name="yst", bufs=2) as yst, \
             tc.tile_pool(name="psS", bufs=2, space="PSUM") as psS, \
             tc.tile_pool(name="psV", bufs=2, space="PSUM") as psV, \
             tc.tile_pool(name="psT", bufs=2, space="PSUM") as psT, \
             tc.tile_pool(name="psY", bufs=2, space="PSUM") as psY:

            wo_seq = [(oi, h) for oi in range(NO) for h in range(NH)]

            def wo_step(state, nsteps):
                for _ in range(nsteps):
                    if state is None or state["pos"] >= len(wo_seq):
                        return
                    oi, h = wo_seq[state["pos"]]
                    state["pos"] += 1
                    if h == 0:
                        if state["wo_next"] is not None:
                            state["wo"] = state["wo_next"]
                        else:
                            wo = woP.tile([P, NH, P], bf16, tag="wo",
                                          name="wo")
                            nc.sync.dma_start(
                                out=wo,
                                in_=WoT_d[:, oi * P:(oi + 1) * P]
                                .rearrange("(hh p) o -> p hh o", p=P))
                            state["wo"] = wo
                        # prefetch next oi's tile
                        if oi + 1 < NO:
                            wo2 = woP.tile([P, NH, P], bf16, tag="wo",
                                           name="wo2")
                            nc.sync.dma_start(
                                out=wo2,
                                in_=WoT_d[:, (oi + 1) * P:(oi + 2) * P]
                                .rearrange("(hh p) o -> p hh o", p=P))
                            state["wo_next"] = wo2
                        else:
                            state["wo_next"] = None
                        state["yp"] = psY.tile([P, TC], f32, tag="yps",
                                               name="yps")
                    nc.tensor.matmul(
                        state["yp"], state["wo"][:, h, :],
                        state["aoT"][:, h, :],
                        start=(h == 0), stop=(h == NH - 1))
                    if h == NH - 1:
                        ys = yst.tile([P, TC], f32, tag="ys", name="ys")
                        nc.vector.tensor_copy(ys, state["yp"])
                        nc.sync.dma_start(
                            out=yT_d[oi * P:(oi + 1) * P,
                                     state["t0"]:state["t0"] + TC],
                            in_=ys)

            NSLAB = 4                # token slabs per chunk for q/k/vg loads
            SLT = TC // NSLAB        # 128 tokens per slab

            def emit_loads(c):
                """DMA loads for chunk c (fast feature-major layout) and the
                vg load. Returns state used by emit_relayout + the group
                loop. The matmul needs token-major contiguous (t,g) columns,
                which the strided DMA can't produce efficiently (2-byte
                gather) — so we DMA feature-major slabs and relayout on the
                vector engines (emit_relayout), one strided copy per slab."""
                t0 = c * TC
                q_sl = []
                k_sl = []
                vg = vgp.tile([P, NG, HD + 2], bf16, tag="vg", name="vg")
                nc.vector.memset(vg[:, :, HD:HD + 1], 1.0)
                for s in range(NSLAB):
                    ts = t0 + s * SLT
                    qs = qkp.tile([P, NH, SLT], bf16, tag="qsl",
                                  name="q_slab", bufs=4)
                    nc.sync.dma_start(
                        out=qs,
                        in_=qT_d[:, ts:ts + SLT]
                        .rearrange("(g p) t -> p g t", p=P))
                    q_sl.append(qs)
                    ks = qkp.tile([P, NH, SLT], bf16, tag="ksl",
                                  name="k_slab", bufs=4)
                    nc.sync.dma_start(
                        out=ks,
                        in_=kT_d[:, ts:ts + SLT]
                        .rearrange("(g p) t -> p g t", p=P))
                    k_sl.append(ks)
                    # vg slab: 16 groups of 8 tokens; partition = (t8, g)
                    nc.sync.dma_start(
                        out=vg[:, s * (SLT // GS):(s + 1) * (SLT // GS),
                               0:HD],
                        in_=vnat_d[ts:ts + SLT, :]
                        .rearrange("(grp t8) (g d) -> (t8 g) grp d",
                                   t8=GS, g=NH))
                q_grp = qkp.tile([P, TC, NH], bf16, tag="qg", name="q_grp")
                k_grp = qkp.tile([P, TC, NH], bf16, tag="kg", name="k_grp")
                return {"q_sl": q_sl, "k_sl": k_sl, "vg": vg,
                        "q_grp": q_grp, "k_grp": k_grp}

            RPIECE = 2               # relayout pieces per slab
            RPT = SLT // RPIECE      # tokens per relayout piece

            def emit_relayout(st, idx):
                """Relayout piece idx (of NSLAB*RPIECE*2) into q_grp/k_grp.
                q pieces go on DVE, k pieces on gpsimd."""
                tensor = idx % 2
                piece = idx // 2
                s, pc = divmod(piece, RPIECE)
                tt = s * SLT + pc * RPT
                if tensor == 0:
                    nc.vector.tensor_copy(
                        st["q_grp"][:, tt:tt + RPT, :],
                        st["q_sl"][s][:, :, pc * RPT:(pc + 1) * RPT]
                        .rearrange("p g t -> p t g"))
                else:
                    nc.gpsimd.tensor_copy(
                        st["k_grp"][:, tt:tt + RPT, :],
                        st["k_sl"][s][:, :, pc * RPT:(pc + 1) * RPT]
                        .rearrange("p g t -> p t g"))

            NREL = NSLAB * RPIECE * 2

            D1 = 2   # AV lag behind scores
            D2 = 3   # transpose-back lag

            prev = None
            loaded = emit_loads(0)
            for i in range(NREL):
                emit_relayout(loaded, i)
            nxt = None
            for c in range(TPC // TC):
                t0 = c * TC
                st = loaded
                q_grp, k_grp, vg = st["q_grp"], st["k_grp"], st["vg"]
                aoT = aop.tile([P, NH, TC], bf16, tag="aoT", name="aoT")

                esm_by_i = {}
                ao_by_i = {}
                for g in range(NG + D2):
                    if g == 4 and c + 1 < TPC // TC:
                        nxt = emit_loads(c + 1)
                    if nxt is not None and 8 <= g < 8 + 2 * NREL \
                            and (g - 8) % 2 == 0:
                        emit_relayout(nxt, (g - 8) // 2)
                        if g == 8 + 2 * NREL - 2:
                            loaded = nxt
                            nxt = None
                    if g < NG:
                        tt = g * GS
                        sc = psS.tile([P, P], f32, tag="scps", name="sc")
                        nc.tensor.matmul(
                            sc,
                            k_grp[:, tt:tt + GS, :]
                            .rearrange("p t h -> p (t h)"),
                            q_grp[:, tt:tt + GS, :]
                            .rearrange("p t h -> p (t h)"),
                            start=True, stop=True)
                        es = mid.tile([P, P], bf16, tag="es", name="es")
                        nc.scalar.activation(
                            out=es, in_=sc,
                            func=mybir.ActivationFunctionType.Exp,
                            scale=SC)
                        esm = mid.tile([P, P], bf16, tag="esm", name="esm")
                        nc.vector.scalar_tensor_tensor(
                            esm, es, 1.0, maskt,
                            op0=AOP.bypass, op1=AOP.mult)
                        esm_by_i[g] = esm
                    if D1 <= g < NG + D1:
                        i = g - D1
                        av = psV.tile([P, HD + 1], f32, tag="avps", name="av")
                        nc.tensor.matmul(av, esm_by_i.pop(i),
                                         vg[:, i, 0:HD + 1],
                                         start=True, stop=True)
                        iv = mid.tile([P, 1], f32, tag="iv", name="iv")
                        nc.vector.reciprocal(iv, av[:, HD:HD + 1])
                        ao = mid.tile([P, HD], bf16, tag="ao", name="ao")
                        nc.vector.tensor_scalar_mul(ao, av[:, 0:HD], iv)
                        ao_by_i[i] = ao
                    if D2 <= g:
                        i = g - D2
                        at = psT.tile([P, P], bf16, tag="atps", name="at")
                        nc.tensor.transpose(at, ao_by_i.pop(i), ident)
                        nc.scalar.copy(
                            aoT[:, :, i * GS:(i + 1) * GS]
                            .rearrange("p h t -> p t h"),
                            at.rearrange("p (t h) -> p t h", t=GS))
                    wo_step(prev, 4)

                wo_step(prev, len(wo_seq))
                prev = {"pos": 0, "aoT": aoT, "t0": t0,
                        "wo": None, "wo_next": None, "yp": None}
            wo_step(prev, len(wo_seq))

    nc.compile()
    return nc


def _get_program():
    if "nc" not in _cached:
        _cached["nc"] = _build_program()
    return _cached["nc"]


def kernel(x, Wq, Wk, Wv, Wo):
    import ml_dtypes
    from concourse.bass_utils import run_bass_kernel_spmd

    bf16 = ml_dtypes.bfloat16
    B, S, H = x.shape
    assert (B * S, H) == (NCORES * TPC, E)
    nc = _get_program()

    xf = np.ascontiguousarray(x.reshape(B * S, H))
    WqT = np.ascontiguousarray(Wq.T).astype(bf16)
    WkT = np.ascontiguousarray(Wk.T).astype(bf16)
    WvT = np.ascontiguousarray(Wv.T).astype(bf16)
    WoT = np.ascontiguousarray(Wo.T).astype(bf16)
    ident = np.eye(P, dtype=bf16)
    mask = np.kron(np.eye(GS, dtype=np.float32),
                   np.ones((NH, NH), dtype=np.float32)).astype(bf16)

    in_maps = []
    for i in range(NCORES):
        xT = np.ascontiguousarray(xf[i * TPC:(i + 1) * TPC, :].T).astype(bf16)
        in_maps.append({"xT": xT, "WqT": WqT, "WkT": WkT,
                        "WvT": WvT, "WoT": WoT, "ident": ident,
                        "mask": mask})

    import os
    trace = bool(int(os.environ.get("BASS_KERNEL_TRACE", "0")))
    res = run_bass_kernel_spmd(nc, in_maps, core_ids=list(range(NCORES)),
                               trace=trace)
    if trace:
        _cached["last_results"] = res
    parts = [res.results[i]["yT"].T for i in range(NCORES)]
    y = np.concatenate(parts, axis=0).reshape(B, S, H)
    return np.ascontiguousarray(y.astype(np.float32))
